# revision 6
# baseline (speedup 1.0000x reference)
"""Trainium2 Bass kernel for the supervoxel erode/edge loss module.

The reference divides a padded [B,X,Y] grid (pad offset 4*sx along x, 4*sy
along y) into 8x8 patches, zeroes the last row/col of the mask channel in
each patch, erodes along both patch axes and sums eroded*edge. The erode
`a*b + (1-a)*a + (1-b)*a` equals `2a - a^2` with a = m(i)*m(i+1), and the
whole module collapses to a global elementwise expression on the grid:

    mt(x,y) = mask[b,x,y,idx] * [(x+4sx)%8 != 7] * [(y+4sy)%8 != 7]
    ax = mt(x,y)*mt(x+1,y); ay = mt(x,y)*mt(x,y+1)   (zero past image edge)
    total = sum_b,x,y ax(2-ax) * ay(2-ay) * edge
    out = loss_old + total / (B * ((X+8)//8) * ((Y+8)//8))

Implementation notes (performance-driven; see measured DMA behavior):
  * Per-SDMA-engine rate caps at ~13 GB/s (so ~210 GB/s/core aggregate);
    the kernel therefore minimizes HBM bytes and keeps all 16 engines fed
    from three queues (two HWDGE rings for the mask + SWDGE for the edge).
  * 2 grid rows per SBUF partition -> 32 KiB contiguous per mask descriptor.
    Tiles stride 240 rows (121 partitions, one-partition overlap so the
    x+1 neighbor product never crosses a tile; 240%8==0 keeps the row
    pattern tile-invariant).
  * Row mask R: contribution rows x with (x+4sx)%8 in {6,7} are dead; both
    land in partitions p = 3-2sx (mod 4), so the edge tile is zeroed and
    those partitions are simply never loaded (also saves 1/4 of the edge
    traffic). No rvec pass needed.
  * Col mask C folds into a strided live-column view on the final fused op
    (in bf16 an 8-col group is exactly one 16B SBUF line, so this is cheap).
  * Compute is dense bf16: ACT extracts the mask channel (strided f32 read,
    cast) and computes P=(ax-1)^2, Q=(ay-1)^2; PE shifts rows via a
    matmul; DVE does 3 muls + a stt + a final stt with fused per-partition
    accumulation. nx*ny = (P-1)*(Q-1), so
    contribution = (P-1) * [(Q-1)*edge], accumulated per partition.
  * Edge is cast f32->bf16 during the SWDGE DMA (free).

Sharding: data-parallel over batch, B/8 images per core; per-core partial
sums are combined on the host (final result is one scalar).
"""

import sys

sys.path.insert(0, "/opt/trn_rl_repo")

import numpy as np

from concourse import bacc, bass, mybir, tile
from concourse.ap import AP
from concourse.bass_utils import run_bass_kernel_spmd

F32 = mybir.dt.float32
BF16 = mybir.dt.bfloat16
N_CORES = 8
SHIFTS = [(0, 0), (1, 0), (0, 1), (1, 1)]


def _col_runs(sy: int, Y: int):
    """Live contribution columns as strided runs (off, ngroups, runlen):
    cols y = off + 8*g + k, k in [0, runlen). Live cols satisfy
    (y+4sy)%8 not in {6,7}; they form one run of 6 per 8-group (mod phase).
    Runs must stay within [0, Y-2] (col Y-1 has no y+1 neighbor)."""
    lo = (6 - 4 * sy) % 8  # first dead col in each 8-group
    if lo == 6:  # sy == 0: live {0..5}
        return [(0, Y // 8, 6)]
    # sy == 1: dead {2,3}; live runs: {4..9} wrapping -> one strided run of 6
    # starting at 4 (g = 0..Y//8-2), plus head {0,1} and tail {Y-4..Y-2}.
    return [(4, Y // 8 - 1, 6), (0, 1, 2), (Y - 4, 1, 3)]


def _build_program(Bc: int, X: int, Y: int, idx: int):
    """Per-core program. Inputs: mask [Bc,X,Y,4] f32, edge [Bc,X,Y,1] f32,
    smat [128,128] bf16, ones [128,1] f32. Output: out [1,1] f32 partial sum.
    """
    sx, sy = SHIFTS[idx]
    assert X % 8 == 0 and Y % 8 == 0
    TS = 240  # grid rows per full tile (120 partitions * 2 rows)
    nfull = (X - 2) // TS  # full tiles (with overlap partition)
    tail_rows = X - TS * nfull  # remaining rows
    assert tail_rows % 2 == 0
    tailp = tail_rows // 2  # partitions in tail tile (no overlap partition)
    nk = nfull + (1 if tailp else 0)
    runs = _col_runs(sy, Y)
    nruns = len(runs)
    NT = Bc * nk * nruns
    pd = (3 - 2 * sx) % 4  # dead-contribution partition residue (mod 4)
    pc = 3 if pd == 1 else 1  # the live odd residue
    row = Y * 4  # f32 elements per grid row

    nc = bacc.Bacc("TRN2", target_bir_lowering=False, debug=False)
    mask_h = nc.dram_tensor("mask", [Bc, X, Y, 4], F32, kind="ExternalInput")
    edge_h = nc.dram_tensor("edge", [Bc, X, Y, 1], F32, kind="ExternalInput")
    smat_h = nc.dram_tensor("smat", [128, 128], BF16, kind="ExternalInput")
    ones_h = nc.dram_tensor("ones", [128, 1], F32, kind="ExternalInput")
    out_h = nc.dram_tensor("out", [1, 1], F32, kind="ExternalOutput")

    with tile.TileContext(nc) as tc:
        with (
            tc.tile_pool(name="mt", bufs=3) as mt_pool,
            tc.tile_pool(name="et", bufs=2) as et_pool,
            tc.tile_pool(name="vd", bufs=2) as vd_pool,
            tc.tile_pool(name="shc", bufs=2) as shc_pool,
            tc.tile_pool(name="ax", bufs=2) as ax_pool,
            tc.tile_pool(name="ay", bufs=2) as ay_pool,
            tc.tile_pool(name="pq", bufs=2) as pq_pool,
            tc.tile_pool(name="ff", bufs=2) as f_pool,
            tc.tile_pool(name="psum", bufs=2, space="PSUM") as ps_pool,
            tc.tile_pool(name="psum1", bufs=1, space="PSUM") as ps1_pool,
            tc.tile_pool(name="const", bufs=1) as c_pool,
        ):
            smat_t = c_pool.tile([128, 128], BF16)
            ones_t = c_pool.tile([128, 1], F32)
            partials = c_pool.tile([128, NT], F32)
            nc.sync.dma_start(smat_t[:], smat_h.ap())
            nc.sync.dma_start(ones_t[:], ones_h.ap())
            nc.vector.memset(partials[:], 0.0)

            def emit_tile(b: int, t: int, hwdge_eng):
                full = t < nfull
                P2 = 121 if full else tailp  # partitions incl. overlap
                cr = 120 if full else tailp  # contribution partitions
                r0 = TS * t  # first grid row of tile
                base = (b * X + r0) * row

                # ---- mask load: [P2, 2, Y, 4] f32, 2 consecutive rows per
                # partition = 32 KiB contiguous per descriptor.
                mt = mt_pool.tile([P2, 2, Y, 4], F32)
                hwdge_eng.dma_start(
                    mt.rearrange("p j y c -> p (j y c)"),
                    AP(mask_h, base, [[2 * row, P2], [1, 2 * row]]),
                )

                # ---- edge load: bf16 cast during SWDGE DMA; dead-row
                # partitions (p%4==pd) stay zero -> folds the row mask R.
                ebase = (b * X + r0) * Y
                et = et_pool.tile([P2 + (-P2) % 4, 2, Y], BF16)
                nc.gpsimd.memset(et[:].rearrange("p j y -> p (j y)"), 0.0)
                ne1 = (P2 + 1) // 2  # even partitions
                nc.gpsimd.dma_start(
                    et.rearrange("(q r) j y -> q r (j y)", r=2)[0:ne1, 0, :],
                    AP(edge_h, ebase, [[4 * Y, ne1], [1, 2 * Y]]),
                )
                ne2 = (P2 - pc + 3) // 4  # odd live partitions
                nc.gpsimd.dma_start(
                    et.rearrange("(u s) j y -> u s (j y)", s=4)[0:ne2, pc, :],
                    AP(edge_h, ebase + 2 * pc * Y, [[8 * Y, ne2], [1, 2 * Y]]),
                )

                # ---- channel extract + cast on ACT: [P2, 2, Y] bf16 dense
                vd = vd_pool.tile([P2, 2, Y], BF16)
                nc.scalar.copy(vd[:], mt[:, :, :, idx])

                # ---- row shift via PE: shB[p] = vd[p+1, 0, :]
                shp = ps_pool.tile([128, Y], F32)
                for c0 in range(0, Y, 512):
                    nc.tensor.matmul(
                        shp[:, c0 : c0 + 512],
                        smat_t[0:P2, :],
                        vd[:, 0, c0 : c0 + 512],
                        start=True,
                        stop=True,
                    )
                shc = shc_pool.tile([cr, Y], BF16)
                nc.vector.tensor_copy(shc[:], shp[0:cr, :])

                # ---- ax pairs: axA=(2p,2p+1) same partition; axB=(2p+1,2p+2)
                ax = ax_pool.tile([cr, 2, Y], BF16)
                nc.vector.tensor_mul(ax[:, 0, :], vd[0:cr, 0, :], vd[0:cr, 1, :])
                nc.vector.tensor_mul(ax[:, 1, :], vd[0:cr, 1, :], shc[0:cr, :])
                # ---- ay pairs: (y, y+1) within each row
                W = Y - 1
                ay = ay_pool.tile([cr, 2, Y], BF16)
                nc.vector.tensor_mul(
                    ay[:, :, 0:W], vd[0:cr, :, 0:W], vd[0:cr, :, 1:Y]
                )

                # ---- P=(ax-1)^2, Q=(ay-1)^2 on ACT
                pt = pq_pool.tile([cr, 2, Y], BF16)
                qt = pq_pool.tile([cr, 2, Y], BF16)
                nc.scalar.activation(
                    pt[:],
                    ax[:],
                    mybir.ActivationFunctionType.Square,
                    bias=1.0,
                    scale=-1.0,
                )
                nc.scalar.activation(
                    qt[:, :, 0:W],
                    ay[:, :, 0:W],
                    mybir.ActivationFunctionType.Square,
                    bias=1.0,
                    scale=-1.0,
                )

                # ---- F=(Q-1)*edge; contribution=(P-1)*F summed over live cols
                ft = f_pool.tile([cr, 2, Y], BF16)
                nc.vector.scalar_tensor_tensor(
                    ft[:, :, 0:W],
                    qt[:, :, 0:W],
                    1.0,
                    et[0:cr, :, 0:W],
                    op0=mybir.AluOpType.subtract,
                    op1=mybir.AluOpType.mult,
                )
                tbase = (b * nk + t) * nruns
                for r, (off, ng, rl) in enumerate(runs):
                    view = lambda tl: tl.rearrange(
                        "p j (g k) -> p j g k", k=8
                    )[:, :, off // 8 : off // 8 + ng, off % 8 : off % 8 + rl]
                    nc.vector.scalar_tensor_tensor(
                        view(qt),
                        view(pt),
                        1.0,
                        view(ft),
                        op0=mybir.AluOpType.subtract,
                        op1=mybir.AluOpType.mult,
                        accum_out=partials[0:cr, tbase + r : tbase + r + 1],
                    )

            for b in range(Bc):
                for t in range(nk):
                    eng = nc.sync if (b * nk + t) % 2 == 0 else nc.scalar
                    emit_tile(b, t, eng)

            # total = sum_p sum_t partials[p, t]
            red = c_pool.tile([128, 1], F32)
            nc.vector.reduce_sum(red[:], partials[:], axis=mybir.AxisListType.X)
            out_ps = ps1_pool.tile([1, 1], F32)
            nc.tensor.matmul(out_ps[:], red[:], ones_t[:], start=True, stop=True)
            out_sb = c_pool.tile([1, 1], F32)
            nc.vector.tensor_copy(out_sb[:], out_ps[:])
            nc.sync.dma_start(out_h.ap(), out_sb[:])

    nc.compile()
    return nc


def _host_consts():
    smat = np.zeros((128, 128), np.float32)
    for p in range(127):
        smat[p + 1, p] = 1.0
    import ml_dtypes

    return smat.astype(ml_dtypes.bfloat16), np.ones((128, 1), np.float32)


def _run(mask, edge, loss_old, idx, trace=False, **build_kwargs):
    B, X, Y, _ = mask.shape
    assert B % N_CORES == 0
    Bc = B // N_CORES

    nc = _build_program(Bc, X, Y, idx, **build_kwargs)
    smat, ones = _host_consts()
    in_maps = [
        {
            "mask": mask[i * Bc : (i + 1) * Bc],
            "edge": edge[i * Bc : (i + 1) * Bc],
            "smat": smat,
            "ones": ones,
        }
        for i in range(N_CORES)
    ]
    res = run_bass_kernel_spmd(nc, in_maps, list(range(N_CORES)), trace=trace)
    total = float(sum(float(res.results[i]["out"][0, 0]) for i in range(N_CORES)))
    n_patch = ((X + 8) // 8) * ((Y + 8) // 8)
    out = np.float32(np.asarray(loss_old, dtype=np.float32) + total / (B * n_patch))
    return np.asarray(out, dtype=np.float32), res


def kernel(resized_image, mask_combined, edge_map, loss_old, mask_index):
    mask = np.ascontiguousarray(np.asarray(mask_combined, dtype=np.float32))
    edge = np.ascontiguousarray(np.asarray(edge_map, dtype=np.float32))
    idx = int(np.asarray(mask_index))
    out, _ = _run(mask, edge, loss_old, idx)
    return out
